# revision 33
# baseline (speedup 1.0000x reference)
"""Trainium2 Bass kernel for nn_BlockSampleFixed_47090021434001.

Reference semantics: for input (16, 64, 64, 64) f32, the output
(65536, 64, 4, 4) satisfies

    out[(b*64 + y)*64 + x, c, i, j] = in[b, c, y+i-3, x+j-2]

(zero outside bounds), with taps (i=3, j>=2) masked to zero — a 16-fold
shifted/zero-padded replication of the input transposed from
channel-major to pixel-major.

Strategy (pure data parallel, 2 batches per NeuronCore, no collectives):
  * Everything runs in bf16 (gate is rel-err < 2e-2, bf16 error ~2e-3);
    host converts in/out, HBM traffic halves.
  * Only the 14 unmasked taps are stored (s = 4i+j for i<3, s=12+j for
    i=3, j<2): 896 output columns per pixel instead of 1024, cutting
    store traffic 12.5%.  The host gather writes the device data into a
    zeroed (.., c, 4, 4) f32 array, which re-creates the two masked-tap
    zero channels.
  * The host sends slab 0 (x-padded raw input, c-major layout) in THREE
    pieces with separate SBUF tiles so early consumers wait only on the
    bytes they read (ranges overlap so no tap copy spans a boundary):
    shift matrices + xx<12 (0.3 MB, unblocks the first matmul and the
    x<9 output tiles), xx in [8,24), and xx in [21,72).
  * Slabs d=1..3 (input shifted down d rows) are built on-device by the
    otherwise-idle TensorEngine: per 8-column chunk, 3 matmuls by shift
    matrices (d=3 first — its consumer tap is first), each into its own
    1-bank PSUM tile from an 8-buffer pool so the pipeline never stalls
    on a pending cast.  The 1x-mode PSUM->SBUF casts go to ACT, except
    d=3/d=1 of the first chunks which go to DVE for ramp latency.
    Batch-boundary zero rows fall out of the zero matrix columns.
    Chunks are built one output tile ahead of their consumers.
  * Per x-tile, 4 tap-group engine copies (one per kernel row i)
    assemble the pixel-major [128, w*896] tile: with the c-major slab
    layout both src and dst have contiguous 4-element j-runs (out col =
    c*14 + 4i + j <- slab xx = x0+x+j+1).  i=0..2 run on DVE (~2.4
    elem/ns for 4-elem runs); the nj=2 row i=3 runs on ACT (DVE
    collapses to ~0.15 elem/ns on 2-elem runs; GPSIMD is ~0.25 always).
  * Tile widths [1,1,2,4,4,8,8,12,12,12]: narrow first tiles get the
    store stream going right behind the input load; mid-size tiles keep
    the store queue from outrunning tap assembly.
  * HBM traffic per core: ~1.4 MB in + 14.7 MB out; the measured data
    phase is bubble-free at the per-core HBM rate (~375 GB/s).

The module also carries two workarounds for the walrus build in this
container, which rejects instructions carrying more than one immediate
semaphore wait ("Too many sync wait commands"): the TileContext final
drain's waits are split over sequencer NOPs, and a serialized-BIR
rewrite moves excess waits from any instruction onto injected
same-engine NoOps.
"""

import json as _json

import numpy as np

import concourse.bass as bass
import concourse.mybir as mybir
import concourse.tile as tile
from concourse.vector_clock import ScopedClock, VectorClock

# ---------------------------------------------------------------------------
# walrus workaround #1: split the TileContext final-drain sem waits over
# several sequencer NOPs (<= 4 clock procs each).


def _split_drain_and_barrier(self, tick_clock, wait_clock):
    gclock = tick_clock.global_clock
    n = len(gclock)
    CHUNK = 4
    for start in range(0, n, CHUNK):
        vec = [0] * n
        nonzero = False
        for p in range(start, min(start + CHUNK, n)):
            t = gclock[p]
            vec[p] = t
            if t:
                nonzero = True
        if not nonzero:
            continue
        nop_inst = self.nc.sync.nop(nofuse=True, hint="drain_wait_split")
        wait_clock.add_sem_waits(nop_inst.ins, ScopedClock({None: VectorClock(vec)}))
    self.nc.sync.drain()
    self.nc.all_engine_barrier()
    popped = self.nc._tile_sem_poison_stack.pop()
    assert popped is self._sem_poison
    self.nc.clear_and_free_semaphores(list(self.sems.allocated().values()))
    self.nc.all_engine_barrier()


# ---------------------------------------------------------------------------
# walrus workaround #2: rewrite serialized BIR so no instruction carries
# more than one immediate sem wait; excess waits go to injected NoOps
# placed immediately before it (engine queues execute in list order).

_WSPLIT_KEEP = 1
_WSPLIT_NOP_CHUNK = 1


def _split_bir_waits(bir_json):
    d = _json.loads(bir_json)
    n_new = 0
    for f in d.get("functions", []):
        for bb in f.get("blocks", []):
            insts = bb.get("instructions", [])
            out = []
            for inst in insts:
                si = inst.get("sync_info")
                waits = (si or {}).get("on_wait") or []
                movable = [w for w in waits if w.get("wait_reg") is None]
                fixed = [w for w in waits if w.get("wait_reg") is not None]
                nop_chunk = _WSPLIT_NOP_CHUNK
                keep_limit = (
                    nop_chunk if inst.get("opcode") == "NoOp" else _WSPLIT_KEEP
                )
                if len(waits) > keep_limit:
                    keep_n = max(0, keep_limit - len(fixed))
                    keep, excess = movable[:keep_n], movable[keep_n:]
                    for i in range(0, len(excess), nop_chunk):
                        n_new += 1
                        out.append(
                            {
                                "debug": inst.get("debug"),
                                "engine": inst["engine"],
                                "ins": [],
                                "outs": [],
                                "name": f"I-wsplit-{n_new}",
                                "opcode": "NoOp",
                                "sync_info": {
                                    "on_update": [],
                                    "on_wait": excess[i:i + nop_chunk],
                                },
                                "text_hint": "wait_split",
                            }
                        )
                    si["on_wait"] = fixed + keep
                out.append(inst)
            bb["instructions"] = out
    enc = _json.dumps(d)
    return enc.encode() if isinstance(bir_json, bytes) else enc


_PATCHED = False


def _install_patches():
    global _PATCHED
    if _PATCHED:
        return
    tile.TileContext._drain_and_barrier = _split_drain_and_barrier

    import concourse.bass_utils as _bu
    import concourse.bass2jax as _b2j

    orig = _bu.compile_bir_kernel
    if not getattr(orig, "_wsplit_wrapped", False):

        def wrapper(bir_json, tmpdir, neff_name="file.neff"):
            return orig(_split_bir_waits(bir_json), tmpdir, neff_name=neff_name)

        wrapper._wsplit_wrapped = True
        _bu.compile_bir_kernel = wrapper
        _b2j.compile_bir_kernel = wrapper

    _PATCHED = True


# ---------------------------------------------------------------------------
# kernel proper

N_CORES = 8
B = 2            # batches per core (16 total / 8 cores)
C = 64
H = 64
W = 64
XX = 72          # padded width: xx = x + 3; cols {0,1,2} and {67..71} zero
R = B * H        # 128 partition rows = (b, y)
SLABF = C * XX   # elems per slab per partition (c-major: addr = c*XX + xx)
S = 14           # stored taps per channel (masked s=14,15 dropped)
COLS = C * S     # 896 output columns per pixel
XB = 8           # xx elems per PE slab-build chunk
NBLK = XX // XB  # 9 chunks per slab
F32 = mybir.dt.float32
BF16 = mybir.dt.bfloat16

# input arrives in three pieces with separate SBUF tiles so early
# consumers wait only on the bytes they read; ranges overlap so no tap
# copy spans a tile boundary:
#   x1a: shift matrices ++ xx in [0, 12)   (chunk 0, d=0 taps x0+w <= 8)
#   x1b: xx in [8, 24)                     (chunks 1-2, d=0 taps to x0+w <= 20)
#   x2:  xx in [12, 72)                    (chunks 3+, remaining d=0 taps)
P1AXX = 12
P1BX0 = 8
P1BXX = 16
P1XX = 24        # chunks below this xx bound come from x1a/x1b
P2X0 = 21        # lowest xx any x2 consumer reads (d=0 tap of the first
P2XX = XX - P2X0 # tile with x0+w+3 >= 24, i.e. xx = x0+1 = 21)
WOFF = 0         # shift matrices at [0, 384) of x1a
P1AOFF = 3 * R
X1AF = 3 * R + C * P1AXX
X1BF = C * P1BXX
X2F = C * P2XX

WIDTHS = [1, 1, 2, 4, 4, 8, 8, 12, 12, 12]
XT2 = 16
NBUF = 5
NEARLY = 3       # chunks built through the latency-optimized early path


def _build_nc():
    import bass_rust

    nc = bass.Bass()
    x1a = nc.dram_tensor("x1a", [R, X1AF], BF16, kind="ExternalInput")
    x1b = nc.dram_tensor("x1b", [R, X1BF], BF16, kind="ExternalInput")
    x2 = nc.dram_tensor("x2", [R, X2F], BF16, kind="ExternalInput")
    out = nc.dram_tensor("out", [B * H * W, COLS], BF16, kind="ExternalOutput")

    with tile.TileContext(nc) as tc:
        with (
            tc.tile_pool(name="inp", bufs=1) as in_pool,
            tc.tile_pool(name="outp", bufs=NBUF) as out_pool,
            tc.psum_pool(name="ps", bufs=8) as ps_pool,
        ):
            xin1 = in_pool.tile([R, X1AF], BF16, tag="xin1", name="xin1")
            xin1b = in_pool.tile([R, X1BF], BF16, tag="xin1b", name="xin1b")
            xin2 = in_pool.tile([R, X2F], BF16, tag="xin2", name="xin2")
            nc.sync.dma_start(xin1[:], x1a[:])
            nc.sync.dma_start(xin1b[:], x1b[:])
            nc.sync.dma_start(xin2[:], x2[:])
            p1ar = xin1[:, P1AOFF:].rearrange(
                "p (c xx) -> p c xx", c=C, xx=P1AXX
            )
            p1br = xin1b[:].rearrange("p (c xx) -> p c xx", c=C, xx=P1BXX)
            p2r = xin2[:].rearrange("p (c xx) -> p c xx", c=C, xx=P2XX)

            # slabs d=1..3 live here at offset (d-1)*SLABF
            t2 = in_pool.tile([R, 3 * SLABF], BF16, tag="t2", name="t2")
            t2r = t2[:].rearrange(
                "p (d c xx) -> p d c xx", d=3, c=C, xx=XX
            )  # (p, d-1, c, xx), xx innermost

            # slab build: for xx-chunk blk and shift d, PSUM[p, (c,xxb)] =
            # sum_k W_d[k, p] * slab0[k, (c,xxb)] = slab0[p-d, ...] with
            # batch-boundary rows zeroed for free (zero matrix columns).
            # All 3 shifts land in one 3-bank PSUM tile; the cast back to
            # SBUF is split into c-halves on DVE and ACT.
            def build_blk(blk):
                if blk == 0:
                    src = p1ar[:, :, 0:XB]
                elif (blk + 1) * XB <= P1XX:
                    src = p1br[:, :, blk * XB - P1BX0:(blk + 1) * XB - P1BX0]
                else:
                    src = p2r[:, :, blk * XB - P2X0:(blk + 1) * XB - P2X0]
                # PSUM->SBUF cast is stuck in 1x mode (fp32 PSUM src).
                # One 1-bank PSUM tile + cast per shift, d=3 first (the
                # i=0 tap reads it), 8 buffers so the matmul pipeline
                # never stalls on a pending cast.  Ramp chunks split the
                # casts across DVE/ACT for latency; later chunks keep DVE
                # free for tap copies.
                for d in (3, 2, 1):
                    ps = ps_pool.tile([R, C * XB], F32, tag="ps",
                                      name=f"ps_{blk}_{d}")
                    nc.tensor.matmul(
                        ps[:],
                        xin1[:, WOFF + (d - 1) * R:WOFF + d * R],
                        src,
                        start=True,
                        stop=True,
                    )
                    dst = t2r[:, d - 1, :, blk * XB:(blk + 1) * XB]
                    psr = ps[:].rearrange("p (c w) -> p c w", c=C, w=XB)
                    if blk < NEARLY and d != 2:
                        nc.vector.tensor_copy(dst, psr)
                    else:
                        nc.scalar.copy(dst, psr)

            assert sum(WIDTHS) == W
            tiles = [
                out_pool.tile(
                    [R, XT2 * COLS], BF16, tag="out_sb", name=f"out_sb_{i}"
                )
                for i in range(len(WIDTHS))
            ]

            blks_built = 0
            x0 = 0
            for xt_i, wdt in enumerate(WIDTHS):
                # build the slab chunks this tile's taps read (xx up to
                # x0 + wdt + 3) plus one tile of lookahead, so the cast ->
                # tap -> store chain of tile k overlaps tile k-1's store
                ahead = WIDTHS[xt_i + 1] if xt_i + 1 < len(WIDTHS) else 0
                need = min(NBLK, (x0 + wdt + ahead + 3) // XB + 1)
                while blks_built < need:
                    build_blk(blks_built)
                    blks_built += 1
                out_sb = tiles[xt_i]
                ov = out_sb[:].rearrange(
                    "p (x c s) -> p x c s", x=XT2, c=C, s=S
                )
                # tap-group copies: one op per kernel row i covers all its
                # j taps with contiguous 4-elem runs on src AND dst:
                #   dst[p, x, c, 4i+j] = slab[d=3-i][p, c, x0+x+j+1]
                for i in range(4):
                    d = 3 - i
                    nj = 4 if i < 3 else 2
                    if d == 0 and x0 + wdt + 3 < P1AXX:
                        src = p1ar[:, :, x0 + 1:x0 + 2]
                        src.ap = bass_rust.VecI64Pair(
                            [[X1AF, R], [1, wdt], [P1AXX, C], [1, nj]]
                        )
                    elif d == 0 and x0 + wdt + 3 < P1XX:
                        src = p1br[:, :, x0 + 1 - P1BX0:x0 + 2 - P1BX0]
                        src.ap = bass_rust.VecI64Pair(
                            [[X1BF, R], [1, wdt], [P1BXX, C], [1, nj]]
                        )
                    elif d == 0:
                        src = p2r[:, :, x0 + 1 - P2X0:x0 + 2 - P2X0]
                        src.ap = bass_rust.VecI64Pair(
                            [[X2F, R], [1, wdt], [P2XX, C], [1, nj]]
                        )
                    else:
                        src = t2r[:, d - 1, :, x0 + 1:x0 + 2]
                        src.ap = bass_rust.VecI64Pair(
                            [[3 * SLABF, R], [1, wdt], [XX, C], [1, nj]]
                        )
                    dst = ov[:, :wdt, :, 4 * i:4 * i + nj]
                    # DVE runs the 4-elem-run copies at ~2.4 elem/ns but
                    # collapses to ~0.15 elem/ns on 2-elem runs (nj=2), so
                    # i=3 always goes to ACT (which also carries the PSUM
                    # casts); i=0..2 go to DVE.  GPSIMD copies measured
                    # ~0.25 elem/ns — never used.
                    if i == 3:
                        nc.scalar.copy(dst, src)
                    else:
                        nc.vector.tensor_copy(dst, src)
                dst = out.rearrange("(r x) n -> r x n", x=W)[:, x0:x0 + wdt, :]
                nc.sync.dma_start(dst, out_sb[:, :wdt * COLS])
                x0 += wdt

    return nc


def _host_prep(xb, wsh):
    """xb: (B, C, H, W) core shard -> (x1a, x1b, x2) bf16 pieces of the
    x-padded c-major slab 0 (see _build_nc for the xx ranges)."""
    import ml_dtypes

    xbt = np.ascontiguousarray(
        xb.transpose(0, 2, 1, 3).astype(ml_dtypes.bfloat16)
    )  # (b, y, c, x)
    t0 = np.zeros((B, H, C, XX), dtype=ml_dtypes.bfloat16)
    t0[:, :, :, 3:3 + W] = xbt
    p1a = np.ascontiguousarray(t0[:, :, :, :P1AXX]).reshape(R, C * P1AXX)
    p1b = np.ascontiguousarray(
        t0[:, :, :, P1BX0:P1BX0 + P1BXX]
    ).reshape(R, X1BF)
    p2 = np.ascontiguousarray(t0[:, :, :, P2X0:]).reshape(R, X2F)
    return np.concatenate([wsh, p1a], axis=1), p1b, p2


def _shift_weights():
    """[k, 3*128] bf16: W_d[k, p]=1 iff p==k+d within the same batch."""
    import ml_dtypes

    w = np.zeros((R, 3, R), dtype=ml_dtypes.bfloat16)
    for d in (1, 2, 3):
        for b in (0, 1):
            for k in range(b * H, (b + 1) * H - d):
                w[k, d - 1, k + d] = 1.0
    return w.reshape(R, 3 * R)


def _in_maps(full):
    wsh = _shift_weights()
    maps = []
    for k in range(N_CORES):
        x1a, x1b, x2 = _host_prep(full[B * k:B * (k + 1)], wsh)
        maps.append({"x1a": x1a, "x1b": x1b, "x2": x2})
    return maps


_NC_CACHE = None


def kernel(inputs):
    """inputs: (16, 64, 64, 64) float32 -> (65536, 64, 4, 4) float32."""
    global _NC_CACHE
    _install_patches()
    from concourse.bass_utils import run_bass_kernel_spmd

    full = np.ascontiguousarray(np.asarray(inputs, dtype=np.float32))
    assert full.shape == (N_CORES * B, C, H, W), full.shape

    if _NC_CACHE is None:
        _NC_CACHE = _build_nc()
    nc = _NC_CACHE

    res = run_bass_kernel_spmd(
        nc, _in_maps(full), core_ids=list(range(N_CORES))
    )
    return _gather(res)


def _gather(res):
    out = np.zeros((N_CORES * B * H * W, C, 16), dtype=np.float32)
    for k in range(N_CORES):
        dev = res.results[k]["out"]  # [B*H*W, 896] bf16
        out[k * B * H * W:(k + 1) * B * H * W, :, :S] = (
            dev.astype(np.float32).reshape(B * H * W, C, S)
        )
    return out.reshape(N_CORES * B * H * W, C, 4, 4)
